# revision 27
# baseline (speedup 1.0000x reference)
"""BiCrossAttention Trainium2 kernel.

Shards the (B=2, H=8) problem across 8 NeuronCores as (batch, head-pair):
core c handles batch c//4 and heads {2*(c%4), 2*(c%4)+1}.  Each core
computes its two heads' QKV projections, both cross-attention branches,
and a partial output projection; the host sums the 4 per-batch partials
and adds the bias.

Device-side layout notes:
  - activations are passed pre-transposed/tiled: xT[p, kc, n] = x[n, kc*128+p]
  - matmuls run in bf16 (1 cyc/row) except the output projection and
    reciprocal broadcast (float32r, the single-pass fp32 mode, 2 cyc/row)
  - scores are computed transposed (simT[j, i]) so exp feeds the attn@V
    matmul directly as a stationary operand
  - the inner loop processes (branch0, head X) and (branch1, head Y)
    together: their K=64 score matmuls occupy disjoint PE row groups
    (partitions 0-63 vs 64-127) and can run concurrently, and one
    Exp instruction covers both members' scores
  - the softmax denominator comes free from an all-ones column appended to
    V; its reciprocal is broadcast across partitions with a K=1 matmul
  - QKV projection chunks and the output projection are interleaved into
    the attention loop as PE filler, keeping the PE dense so the HAM clock
    gate stays at full rate
  - alpha gating is folded into the V weights on the host
"""

import sys
import types

import numpy as np

for _p in ("/opt/trn_rl_repo",):
    if _p not in sys.path:
        sys.path.append(_p)

# Register the axon NTFF profile hook if the image's antenv lacks it (needed
# only when tracing; harmless otherwise).
try:
    import antenv

    if "antenv.axon_hooks" not in sys.modules:
        try:
            import antenv.axon_hooks  # noqa: F401
        except ImportError:
            _hooks = types.ModuleType("antenv.axon_hooks")
            _hook_holder = [None]
            _hooks.set_axon_ntff_profile_hook = lambda h: _hook_holder.__setitem__(0, h)
            _hooks.get_axon_ntff_profile_hook = lambda: _hook_holder[0]
            sys.modules["antenv.axon_hooks"] = _hooks
            antenv.axon_hooks = _hooks
            try:
                from trn_agent_boot.trn_boot import _ntff_profile_via_ctypes

                _hooks.set_axon_ntff_profile_hook(
                    _ntff_profile_via_ctypes("/opt/axon/libaxon_pjrt.so")
                )
            except Exception:
                pass
except Exception:
    pass

import ml_dtypes
import concourse.bacc as bacc
import concourse.mybir as mybir
import concourse.tile as tile
from concourse import bass_utils
from concourse.masks import make_identity

F32 = mybir.dt.float32
F32R = mybir.dt.float32r
BF16 = mybir.dt.bfloat16

_NP = {F32: np.float32, F32R: np.float32, BF16: ml_dtypes.bfloat16}

# Full problem constants
B, N, QD, CD, H, DH = 2, 2048, 1024, 1024, 8, 64
INNER = H * DH
SCALE = DH**-0.5
N_CORES = 8
HG = 4  # head-groups (of 2 heads) per batch


class Cfg:
    def __init__(self, n=N, d=QD, dt_proj=BF16, dt_attn=BF16, dt_out=BF16):
        self.N = n          # sequence length
        self.D = d          # model dim (= QD = CD)
        self.KC = d // 128  # contraction chunks for projections
        self.ISLAB = min(512, n)   # attention i-slab / projection i-chunk
        self.NJC = n // 128  # j chunks (128 keys each)
        self.dt_proj = dt_proj
        self.dt_attn = dt_attn
        self.dt_out = dt_out


def build_nc(cfg: Cfg):
    """Builds the single-core program (SPMD across all 8 cores)."""
    nc = bacc.Bacc("TRN2", target_bir_lowering=False, debug=False)
    KC, Nn, D = cfg.KC, cfg.N, cfg.D
    ISLAB, NJC = cfg.ISLAB, cfg.NJC
    NSL = Nn // ISLAB
    NCH = Nn // ISLAB  # projection chunks per tensor
    DTP, DTA, DTO = cfg.dt_proj, cfg.dt_attn, cfg.dt_out

    NCH_ = Nn // min(512, Nn)
    xT = nc.dram_tensor(
        "xT", [NCH_, 128, KC, min(512, Nn)], DTP, kind="ExternalInput"
    ).ap()
    cT = nc.dram_tensor(
        "cT", [NCH_, 128, KC, min(512, Nn)], DTP, kind="ExternalInput"
    ).ap()
    wd = {
        name: nc.dram_tensor(name, [128, KC, 128], DTP, kind="ExternalInput").ap()
        for name in ("wq1", "wk1", "wv1", "wq2", "wk2", "wv2")
    }
    wout_d = nc.dram_tensor("wout", [128, D], DTO, kind="ExternalInput").ap()
    y_d = nc.dram_tensor("y", [Nn, D], F32, kind="ExternalOutput").ap()

    with tile.TileContext(nc) as tc:
        with (
            tc.tile_pool(name="const", bufs=1) as cpool,
            tc.tile_pool(name="qkv", bufs=1) as qkvpool,
            tc.tile_pool(name="vaug", bufs=1) as vaugpool,
            tc.tile_pool(name="outp", bufs=1) as outpool,
            tc.tile_pool(name="slab", bufs=8) as slabpool,
            tc.tile_pool(name="exp", bufs=6) as exppool,
            tc.tile_pool(name="tmp", bufs=4) as tmppool,
            tc.tile_pool(name="ysb", bufs=3) as ypool,
            tc.tile_pool(name="sim", bufs=2, space="PSUM") as simpool,
            tc.tile_pool(name="acc", bufs=2, space="PSUM") as accpool,
            tc.tile_pool(name="util", bufs=2, space="PSUM") as utilpool,
        ):
            # ---- constants ----
            # first input slabs are queued before the weights so the
            # projection pipeline starts as early as possible
            _early = []
            for ch in range(min(1, Nn // ISLAB)):
                for stream, src_ap in (("2", cT), ("1", xT)):
                    xs = slabpool.tile(
                        [128, KC, ISLAB], DTP, tag="xs", name=f"xs{stream}{ch}"
                    )
                    nc.sync.dma_start(out=xs[:], in_=src_ap[ch])
                    _early.append((stream, ch, xs))
            w_sb = {}
            for name in ("wq2", "wk2", "wv2", "wq1", "wk1", "wv1"):
                w_sb[name] = cpool.tile([128, KC, 128], DTP, tag=name, name=name)
                nc.sync.dma_start(out=w_sb[name][:], in_=wd[name])
            wout_sb = cpool.tile([128, D], DTO, tag="wout")
            nc.sync.dma_start(out=wout_sb[:], in_=wout_d)
            ident_f32 = cpool.tile([128, 128], F32, tag="ident_f32")
            make_identity(nc, ident_f32[:])
            ident = cpool.tile([128, 128], DTA, tag="ident")
            nc.vector.tensor_copy(out=ident[:], in_=ident_f32[:])
            ones_f32 = cpool.tile([128, 64], F32, tag="ones_f32")
            nc.vector.memset(ones_f32[:], 1.0)
            ones_sb = cpool.tile([128, 64], F32R, tag="ones")
            nc.vector.tensor_copy(out=ones_sb[:], in_=ones_f32[:])

            proj = {}
            for name in ("q1", "k1", "v1", "q2", "k2", "v2"):
                proj[name] = qkvpool.tile([128, Nn], DTA, tag=name, name=name)
            SRC = {"1": xT, "2": cT}

            vaug = {}
            for br, h in ((0, 0), (0, 1), (1, 0), (1, 1)):
                vaug[(br, h)] = vaugpool.tile(
                    [128, NJC, 65], DTA, tag=f"vaug{br}{h}", name=f"vaug{br}{h}"
                )
                nc.vector.tensor_copy(
                    out=vaug[(br, h)][:, :, 64],
                    in_=ones_f32[:, 0:1].to_broadcast((128, NJC)),
                )

            # outT rows 0..63 = head 0 (written directly); head 1 staged in
            # outTB then DMA-shifted into rows 64..127 (compute engines are
            # lane-aligned and cannot move data across partitions; DMA can).
            outT = outpool.tile([128, Nn], DTO, tag="outT")
            outTB = outpool.tile([64, Nn], DTO, tag="outTB")

            # ---- streamed projection chunks (PE filler inside attention) ----
            chunk_done = set()
            xs_tiles = {}
            for stream, ch, xs in _early:
                chunk_done.add(("src", stream, ch))
                xs_tiles[(stream, ch)] = xs

            def ensure_src_slab(stream, ch):
                if ("src", stream, ch) in chunk_done:
                    return
                chunk_done.add(("src", stream, ch))
                xs = slabpool.tile(
                    [128, KC, ISLAB], DTP, tag="xs", name=f"xs{stream}{ch}"
                )
                nc.sync.dma_start(out=xs[:], in_=SRC[stream][ch])
                xs_tiles[(stream, ch)] = xs

            def ensure_chunk(pname, ch):
                """Project chunk ch (ISLAB wide) of tensor pname."""
                if (pname, ch) in chunk_done or ch >= NCH:
                    return
                chunk_done.add((pname, ch))
                stream = pname[1]
                ensure_src_slab(stream, ch)
                xs = xs_tiles[(stream, ch)]
                pp = utilpool.tile([128, ISLAB], F32, tag="util", name=f"pp{pname}{ch}")
                wt = w_sb["w" + pname]
                for kc in range(KC):
                    nc.tensor.matmul(
                        pp[:], wt[:, kc, :], xs[:, kc, :],
                        start=(kc == 0), stop=(kc == KC - 1),
                    )
                nc.vector.tensor_copy(
                    out=proj[pname][:, ch * ISLAB : (ch + 1) * ISLAB], in_=pp[:]
                )

            STREAM_PROJ = {"1": ("q1", "k1", "v1"), "2": ("q2", "k2", "v2")}

            def ensure_stream_chunk(stream, ch):
                """All three projections of one ISLAB chunk of a stream, so
                the source slab tile's lifetime is one tight burst."""
                for pname in STREAM_PROJ[stream]:
                    ensure_chunk(pname, ch)

            def ensure_vaug(br, ch):
                """Transpose chunk ch of v{br+1} into the vaug tiles."""
                if ("vaug", br, ch) in chunk_done or ch >= NCH:
                    return
                chunk_done.add(("vaug", br, ch))
                vname = "v1" if br == 0 else "v2"
                ensure_chunk(vname, ch)
                vt = proj[vname]
                for jc in range(ch * (ISLAB // 128), (ch + 1) * (ISLAB // 128)):
                    pt = utilpool.tile([128, 512], DTA, tag="util", name=f"pt{br}{jc}")
                    nc.tensor.transpose(
                        pt[:, 0:128], vt[:, jc * 128 : (jc + 1) * 128], ident[:]
                    )
                    nc.vector.tensor_copy(
                        out=vaug[(br, 0)][:, jc, 0:64], in_=pt[:, 0:64]
                    )
                    nc.vector.tensor_copy(
                        out=vaug[(br, 1)][:, jc, 0:64], in_=pt[:, 64:128]
                    )

            # Deferred PE work (output projection chunks, late q-projection
            # chunks) is queued and pumped one job per j-chunk so it fills PE
            # slack without delaying the score matmuls that feed ACT.
            jobs = []

            def pump(n=1):
                for _ in range(min(n, len(jobs))):
                    jobs.pop(0)()

            def queue_outproj(sl):
                ocw = min(512, D)
                nocs = D // ocw
                for ic in range(sl * (ISLAB // 128), (sl + 1) * (ISLAB // 128)):
                    ysb = ypool.tile([128, D], F32, tag="ysb", name=f"ysb{ic}")

                    def job(ic=ic, ysb=ysb, oc=0):
                        icsl = slice(ic * 128, (ic + 1) * 128)
                        ocsl = slice(oc * ocw, (oc + 1) * ocw)
                        py = utilpool.tile(
                            [128, ISLAB], F32, tag="util", name=f"py{ic}{oc}"
                        )
                        nc.tensor.matmul(
                            py[:, 0:ocw], outT[:, icsl], wout_sb[:, ocsl],
                            start=True, stop=True,
                        )
                        nc.vector.tensor_copy(out=ysb[:, ocsl], in_=py[:, 0:ocw])
                        nc.sync.dma_start(
                            out=y_d[ic * 128 : (ic + 1) * 128, ocsl],
                            in_=ysb[:, ocsl],
                        )

                    for oc in range(nocs):
                        jobs.append(lambda ic=ic, ysb=ysb, oc=oc: job(ic, ysb, oc))

            # pair p=0: (br0, head0) + (br1, head1); p=1: (br0, head1) + (br1, head0)
            # Each member's K=64 score matmuls sit on its head's PE row group,
            # so the two members' matmuls are independent and can overlap.
            PAIRS = (((0, 0), (1, 1)), ((0, 1), (1, 0)))
            JPC = ISLAB // 128  # j-chunks per projection chunk

            for p, members in enumerate(PAIRS):
                for sl in range(NSL):
                    i0 = sl * ISLAB
                    # safety: everything this slab needs (idempotent)
                    if p == 0 and sl == 0:
                        for t in ("q2", "k1", "q1", "k2"):
                            ensure_chunk(t, 0)
                    else:
                        ensure_chunk("q2", sl)
                        ensure_chunk("q1", sl)
                        for ch in range(NCH):
                            for t in ("k2", "k1"):
                                ensure_chunk(t, ch)
                            ensure_vaug(0, ch)
                            ensure_vaug(1, ch)

                    accs = {}
                    for br, h in members:
                        accs[(br, h)] = accpool.tile(
                            [128, ISLAB], F32, tag="acc", name=f"acc{br}{h}"
                        )
                    for jc in range(NJC):
                        if p == 0 and sl == 0:
                            # stream remaining k/v chunks just ahead of use
                            nxt = jc // JPC + 1
                            if jc % JPC == 0 and nxt < NCH:
                                for t in ("k2", "k1"):
                                    ensure_chunk(t, nxt)
                                ensure_vaug(0, nxt)
                                ensure_vaug(1, nxt)
                        if p == 0 and jc in (8, 10) and sl + 1 < NSL:
                            t = "q2" if jc == 8 else "q1"
                            jobs.insert(0, lambda t=t, c=sl + 1: ensure_chunk(t, c))
                        pump(1)

                        jsl = slice(jc * 128, (jc + 1) * 128)
                        simP = simpool.tile([128, 2, ISLAB], F32, tag="sim")
                        for m, (br, h) in enumerate(members):
                            q = proj["q2"] if br == 0 else proj["q1"]
                            k = proj["k1"] if br == 0 else proj["k2"]
                            rs = slice(h * 64, h * 64 + 64)
                            nc.tensor.matmul(
                                simP[:, m, :], k[rs, jsl], q[rs, i0 : i0 + ISLAB],
                                start=True, stop=True, tile_position=(h * 64, 0),
                            )
                        if p == 0 and sl == 0 and jc == 0:
                            ensure_vaug(0, 0)
                            ensure_vaug(1, 0)
                        expP = exppool.tile([128, 2, ISLAB], DTA, tag="exp")
                        nc.scalar.activation(
                            expP[:], simP[:],
                            mybir.ActivationFunctionType.Exp, scale=SCALE,
                        )
                        for m, (br, h) in enumerate(members):
                            nc.tensor.matmul(
                                accs[(br, h)][0:65, :], vaug[(br, h)][:, jc, :],
                                expP[:, m, :],
                                start=(jc == 0), stop=(jc == NJC - 1),
                            )

                    # normalize members: out = acc[:64] / acc[row 64]
                    isl_ = slice(i0, i0 + ISLAB)
                    for br, h in members:
                        acc = accs[(br, h)]
                        ot = outT if h == 0 else outTB
                        rcpf = tmppool.tile([128, ISLAB], F32, tag="rcpf")
                        # pull vals + recip out of PSUM first so the acc slot
                        # frees quickly for the next slab's accumulators;
                        # the custom-DVE reciprocal mishandles nonzero base
                        # partitions, so run on a base-0 slice covering row 64
                        dst = ot[0:64, isl_]
                        if p != 0:
                            tmp = tmppool.tile([64, ISLAB], F32, tag="tmp")
                            dst = tmp[0:64, :]
                        nc.vector.tensor_copy(out=dst, in_=acc[0:64, :])
                        nc.vector.reciprocal_approx_fast(
                            out=rcpf[0:65, :], in_=acc[0:65, :]
                        )
                        rcp = tmppool.tile([128, ISLAB], F32R, tag="rcpr")
                        nc.vector.tensor_copy(
                            out=rcp[64:65, :], in_=rcpf[64:65, :]
                        )
                        bc = utilpool.tile(
                            [128, ISLAB], F32, tag="util", name=f"bc{br}{h}"
                        )
                        nc.tensor.matmul(
                            bc[0:64, :], ones_sb[64:65, :], rcp[64:65, :],
                            start=True, stop=True,
                        )
                        nc.vector.tensor_mul(out=dst, in0=dst, in1=bc[0:64, :])
                        if p != 0:
                            nc.vector.tensor_add(
                                out=ot[0:64, isl_], in0=ot[0:64, isl_],
                                in1=dst,
                            )
                    if p == 1:
                        # both branches final for this slab: shift head 1 into
                        # outT rows 64..127 and queue its output projection
                        nc.sync.dma_start(
                            out=outT[64:128, isl_], in_=outTB[0:64, isl_]
                        )
                        queue_outproj(sl)
            pump(len(jobs))

    nc.compile()
    return nc


_CACHE = {}
_ACTIVE_CFG = Cfg()


def _get_nc():
    if "nc" not in _CACHE:
        _CACHE["nc"] = build_nc(_ACTIVE_CFG)
    return _CACHE["nc"]


def _tile_kpart(a, dt):
    """[K, M] -> [128, K//128, M] with element (p, kc, m) = a[kc*128+p, m]."""
    k, m = a.shape
    return np.ascontiguousarray(
        a.reshape(k // 128, 128, m).transpose(1, 0, 2)
    ).astype(_NP[dt])


def make_in_maps(x, context, Wq1, Wk1, Wv1, Wq2, Wk2, Wv2, alpha_attn, Wout, bout):
    cfg = _ACTIVE_CFG
    alpha = float(1.0 / (1.0 + np.exp(-np.float64(alpha_attn))))
    Wv1s = np.asarray(Wv1, np.float32) * np.float32(alpha)
    Wv2s = np.asarray(Wv2, np.float32) * np.float32(1.0 - alpha)

    def _chunked(a):
        t = _tile_kpart(a, cfg.dt_proj)  # [128, KC, N]
        w = min(512, cfg.N)
        return np.ascontiguousarray(
            t.reshape(128, cfg.KC, cfg.N // w, w).transpose(2, 0, 1, 3)
        )

    xT = [_chunked(np.asarray(x[b], np.float32).T) for b in range(B)]
    cT = [_chunked(np.asarray(context[b], np.float32).T) for b in range(B)]

    in_maps = []
    for c in range(N_CORES):
        b, hg = c // HG, c % HG
        cols = slice(hg * 128, (hg + 1) * 128)
        in_maps.append(
            {
                "xT": xT[b],
                "cT": cT[b],
                "wq1": _tile_kpart(np.asarray(Wq1, np.float32)[:, cols], cfg.dt_proj),
                "wk1": _tile_kpart(np.asarray(Wk1, np.float32)[:, cols], cfg.dt_proj),
                "wv1": _tile_kpart(Wv1s[:, cols], cfg.dt_proj),
                "wq2": _tile_kpart(np.asarray(Wq2, np.float32)[:, cols], cfg.dt_proj),
                "wk2": _tile_kpart(np.asarray(Wk2, np.float32)[:, cols], cfg.dt_proj),
                "wv2": _tile_kpart(Wv2s[:, cols], cfg.dt_proj),
                "wout": np.ascontiguousarray(
                    np.asarray(Wout, np.float32)[cols, :]
                ).astype(_NP[cfg.dt_out]),
            }
        )
    return in_maps


def run_device(in_maps, trace=False, tmpdir=None):
    nc = _get_nc()
    return bass_utils.run_bass_kernel_spmd(
        nc, in_maps, core_ids=list(range(N_CORES)), trace=trace, tmpdir=tmpdir
    )


def kernel(x, context, Wq1, Wk1, Wv1, Wq2, Wk2, Wv2, alpha_attn, Wout, bout):
    in_maps = make_in_maps(
        x, context, Wq1, Wk1, Wv1, Wq2, Wk2, Wv2, alpha_attn, Wout, bout
    )
    res = run_device(in_maps)
    bout32 = np.asarray(bout, np.float32)
    out = np.empty((B, N, QD), np.float32)
    for b in range(B):
        acc = res.results[b * HG]["y"].astype(np.float32).copy()
        for hg in range(1, HG):
            acc += res.results[b * HG + hg]["y"]
        out[b] = acc + bout32[None, :]
    return out
